# revision 40
# baseline (speedup 1.0000x reference)
"""Trainium2 Bass kernel for nn_Attention_38491496907192.

LayerNorm -> QKV projection -> cosine-sim causal attention (8 heads) -> out
projection, for x [2, 2048, 1024], w_qkv [1024, 1536], w_out [512, 1024].

Sharding (8 NeuronCores): core i handles batch i//4 and head pair
(i%4)*2 .. +2 (data parallel over batch, tensor parallel over heads:
w_qkv split column-wise by head, w_out row-wise). Each core emits a
partial [2048, 1024] output; the host sums the 4 partials per batch.

Per-core pipeline (fp16 matmul operands, fp32 PSUM):
  1. LN stats via ones-vector matmuls on PE over x^T chunks (+ DVE/ACT
     squares), pipelined per 512-column chunk. The normalize pass is FOLDED
     INTO the projections: two extra contraction rows (-mu with column-sum
     weights, sd=(var+eps)^0.5 with bias weights) make the projection PSUM
     equal q/rsig exactly; q and k are scale-invariant under
     l2-normalization so they never need a normalize multiply, and V gets
     one per-partition rsig scale during its PSUM->SBUF copy.
  2. All transcendentals use ONE activation table
     (natural_log_exp_and_others: ln/exp/copy/square), so the scalar engine
     never reloads tables. Inverse l2 norms are exp(-0.5*ln(|.|^2+eps))
     evaluated in column layout [128,16] (cheap), moved between row and
     column form with small DRAM round-trip DMAs, and partition-broadcast
     loaded straight from DRAM.
  3. Attention per (q-megablock j, head h) in S^T layout [k, q]: scores for
     k-block pairs in [128,1024] PSUM tiles, exp(8*s) per pair on ACT
     (diagonal pairs slice off never-needed columns), causal masking via
     affine_select on gpsimd, P@V accumulated over k-blocks into [65,512]
     (row 64 = softmax denominator via interleaved ones-columns in V).
  4. Denominator reciprocals are broadcast from DRAM and folded into the
     PV PSUM->SBUF copy (o^T * 1/L), writing both head halves of a
     [128,512] o^T tile (cross-partition-base writes); out-projection is a
     single 128-contraction matmul per output half; y written as fp16.

Cosine-sim attention is bounded (|8s| <= 8) so softmax needs no row-max.
"""

import sys

sys.path.insert(0, "/opt/trn_rl_repo")

import numpy as np
from contextlib import ExitStack

import concourse.bass as bass
import concourse.tile as tile
from concourse import bacc, mybir
from concourse import bass_utils

AF = mybir.ActivationFunctionType
OP = mybir.AluOpType
F16 = mybir.dt.float16
F32 = mybir.dt.float32

B, N, DIM = 2, 2048, 1024
H, D = 8, 64
P = 128
NBLK = N // P            # 16 row blocks
KT = DIM // P            # 8 contraction tiles
QM = N // 512            # 4 query megablocks
N_CORES = 8
LN_EPS = 1e-5
L2_EPS = 1e-9
SCALE = 8.0

_CACHE = {}


def build_nc(stop_after=99):
    nc = bacc.Bacc("TRN2", target_bir_lowering=False, debug=False,
                   num_devices=N_CORES)
    xT_d = nc.dram_tensor("xT", [DIM, N], F16, kind="ExternalInput").ap()
    wqkv_d = nc.dram_tensor("wqkv", [P, KT, 384], F16, kind="ExternalInput").ap()
    waug_d = nc.dram_tensor("waug", [2, 384], F16, kind="ExternalInput").ap()
    wout_d = nc.dram_tensor("wout", [P, DIM], F16, kind="ExternalInput").ap()
    y_d = nc.dram_tensor("y", [N, DIM], F16, kind="ExternalOutput").ap()
    # (no DRAM scratch: row<->column moves use PE transposes; broadcasts use
    # gpsimd partition_broadcast. DRAM round-trips fail NEFF load here.)

    with tile.TileContext(nc) as tc, ExitStack() as ctx:
        const = ctx.enter_context(tc.tile_pool(name="const", bufs=1))
        wq_sb = const.tile([P, KT, 384], F16, name="wq")
        waug_sb = const.tile([2, 384], F16, name="waug")
        wout_sb = const.tile([P, DIM], F16, name="wout")
        ones_c = const.tile([P, 1], F16, name="ones")
        zero_c = const.tile([P, 1], F32, name="zero")
        ln8_c = const.tile([P, 1], F32, name="ln8")
        eps_l2 = const.tile([P, 1], F32, name="epsl2")
        eps_ln = const.tile([1, 1], F32, name="epsln")
        xTs = [const.tile([P, N], F16, name=f"xts{kt}") for kt in range(KT)]
        xaug = const.tile([2, N], F16, name="xaug")
        qT = const.tile([P, N], F16, name="qT")
        kT = const.tile([P, N], F16, name="kT")
        V = const.tile([P, NBLK, 2 * (D + 1)], F16, name="V")
        rsig_col = const.tile([P, NBLK], F32, name="rsigc")
        rkcol = const.tile([P, QM, 8], F32, name="rkcol")
        onef = const.tile([1, 1], F32, name="onef")

        nc.vector.memset(onef[:], 1.0)
        nc.vector.memset(ones_c[:], 1.0)
        nc.vector.memset(zero_c[:], 0.0)
        nc.vector.memset(ln8_c[:], float(np.log(SCALE)))
        nc.vector.memset(eps_l2[:], L2_EPS)
        nc.vector.memset(eps_ln[:], LN_EPS)
        nc.vector.memset(V[:, :, D:D + 1], 1.0)
        nc.vector.memset(V[:, :, 2 * D + 1:2 * D + 2], 1.0)

        # ---- Phase A: LayerNorm stats in transposed layout ----
        # Sums over dim are partition reductions: ones-vector matmuls on PE
        # accumulated over the 8 contraction chunks in PSUM. Squares split
        # DVE/ACT. The mu/var/sd tail is pipelined per 512-column chunk.
        statp = ctx.enter_context(tc.tile_pool(name="stat", bufs=1))
        lnvrow = statp.tile([1, N], F32, name="lnvrow")
        musq = statp.tile([1, N], F32, name="musq")
        varrow = statp.tile([1, N], F32, name="varrow")
        sdrow = statp.tile([1, N], F16, name="sdrow")
        with tc.tile_pool(name="xsq", bufs=2) as xsqp, \
             tc.tile_pool(name="psS", bufs=1, space="PSUM") as psS:
            for kt in range(KT):
                nc.sync.dma_start(xTs[kt][:], xT_d[kt * P:(kt + 1) * P, :])
            nc.sync.dma_start(wq_sb[:], wqkv_d[:])
            nc.sync.dma_start(waug_sb[:], waug_d[:])
            nc.sync.dma_start(wout_sb[:], wout_d[:])
            pstat = [[psS.tile([1, 512], F32, name=f"ps_{c}_{s}")
                      for s in range(2)] for c in range(QM)]
            for kt in range(KT):
                sq = xsqp.tile([P, N], F16, name="sq")
                if kt < 2:
                    nc.scalar.activation(sq[:], xTs[kt][:], AF.Square)
                else:
                    nc.vector.tensor_tensor(sq[:], xTs[kt][:], xTs[kt][:],
                                            OP.mult)
                for c in range(QM):
                    cs = slice(c * 512, (c + 1) * 512)
                    nc.tensor.matmul(pstat[c][0][:], lhsT=ones_c[:],
                                     rhs=xTs[kt][:, cs],
                                     start=(kt == 0), stop=(kt == KT - 1))
                    nc.tensor.matmul(pstat[c][1][:], lhsT=ones_c[:],
                                     rhs=sq[:, cs],
                                     start=(kt == 0), stop=(kt == KT - 1))
            # per chunk: -mu -> mu^2 -> var -> ln -> sd (pipelines across
            # DVE/ACT; sd written straight into aug row 1 cross-partition)
            for c in range(QM):
                cs = slice(c * 512, (c + 1) * 512)
                nc.vector.tensor_scalar(out=xaug[0:1, cs], in0=pstat[c][0][:],
                                        scalar1=-1.0 / DIM, scalar2=None,
                                        op0=OP.mult)
                nc.scalar.activation(musq[0:1, cs], xaug[0:1, cs], AF.Square)
                nc.vector.scalar_tensor_tensor(
                    out=varrow[0:1, cs], in0=pstat[c][1][:],
                    scalar=1.0 / DIM, in1=musq[0:1, cs],
                    op0=OP.mult, op1=OP.subtract)
                nc.scalar.activation(lnvrow[0:1, cs], varrow[0:1, cs], AF.Ln,
                                     bias=eps_ln[:])
                with nc.allow_low_precision(reason="sd ~1, fp16 fine"):
                    nc.scalar.activation(sdrow[0:1, cs], lnvrow[0:1, cs],
                                         AF.Exp, bias=0.0, scale=0.5)
                # engine writes must start at a 32-aligned partition; reach
                # aug row 1 via DMA instead
                nc.sync.dma_start(xaug[1:2, cs], sdrow[0:1, cs])
        # ---- Phases B/C: per-megablock wavefront ----
        psQ = ctx.enter_context(tc.tile_pool(name="psQ", bufs=2, space="PSUM"))
        psA = ctx.enter_context(tc.tile_pool(name="psA", bufs=2, space="PSUM"))
        psO = ctx.enter_context(tc.tile_pool(name="psO", bufs=2, space="PSUM"))
        psY = ctx.enter_context(tc.tile_pool(name="psY", bufs=2, space="PSUM"))
        sqp = ctx.enter_context(tc.tile_pool(name="sq", bufs=2))
        rowp = ctx.enter_context(tc.tile_pool(name="row", bufs=4))
        bcp = ctx.enter_context(tc.tile_pool(name="bc", bufs=3))
        rlp = ctx.enter_context(tc.tile_pool(name="rl", bufs=4))
        epool = ctx.enter_context(tc.tile_pool(name="ep", bufs=3))
        ocp = ctx.enter_context(tc.tile_pool(name="oc", bufs=3))
        outp = ctx.enter_context(tc.tile_pool(name="out", bufs=3))

        # rsig = (var+eps)^-0.5 in column form for the V scale: move the
        # ln(var) row into a [128,16] PSUM column tile with rank-1 matmuls
        # (out[p,0] = row[0,p] * 1), then one exp.
        lnvps = psQ.tile([P, NBLK], F32, name="pq")
        for c in range(NBLK):
            nc.tensor.matmul(lnvps[:, c:c + 1],
                             lhsT=lnvrow[0:1, c * P:(c + 1) * P],
                             rhs=onef[:], start=True, stop=True)
        nc.scalar.activation(rsig_col[:], lnvps[:], AF.Exp, bias=0.0,
                             scale=-0.5)

        # ---- Phase B: projections + norms for ALL megablocks ----
        # Emitted before any attention so the scalar engine front-loads the
        # ln ops and the norm DMA round-trips resolve during projection time.
        # q/k projection PSUM tiles borrow C-phase pool slots (by tag) for
        # j>0: their main accumulations then all run during the load/stats
        # phase (C-phase first uses come later), keeping PE saturated until
        # xaug lands.
        projpool = [(psQ, "pq"), (psA, "ps1"), (psO, "po"), (psY, "py")]
        for j in range(QM):
            js = slice(j * 512, (j + 1) * 512)
            # -- q^T / k^T projection (raw, LN folded via aug rows) --
            for ti, T in enumerate((qT, kT)):
                c0 = ti * 128
                pool, tag = projpool[j]
                pq = pool.tile([P, 512], F32, name=tag)
                for kt in range(KT):
                    nc.tensor.matmul(pq[:], lhsT=wq_sb[:, kt, c0:c0 + 128],
                                     rhs=xTs[kt][:, js],
                                     start=(kt == 0), stop=False)
                nc.tensor.matmul(pq[:], lhsT=waug_sb[:, c0:c0 + 128],
                                 rhs=xaug[:, js], start=False, stop=True)
                with nc.allow_low_precision(reason="fp16 operand prep"):
                    nc.vector.tensor_copy(T[:, js], pq[:])
            # -- inverse l2 norms: exp(-0.5*ln(|.|^2+eps)), one ACT table --
            # q side: ln+exp in row form, partition-broadcast, multiplied
            # into qT. k side: ln rows PE-transposed to a PSUM column tile;
            # 8/|k| stays in column form as the attention exp's scale.
            rqb = bcp.tile([P, 1024], F16, name="rqb")
            lncol = psY.tile([P, 16], F32, name="py")
            for ti, T in enumerate((qT, kT)):
                sq = sqp.tile([P, 512], F16, name="sqv")
                nc.vector.tensor_tensor(sq[:], T[:, js], T[:, js], OP.mult)
                for h in range(2):
                    hs = slice(h * D, (h + 1) * D)
                    pn = psQ.tile([1, 512], F32, name="pq")
                    nc.tensor.matmul(pn[:], lhsT=ones_c[hs, :], rhs=sq[hs, :],
                                     start=True, stop=True)
                    lnr = rowp.tile([1, 512], F32, name="lnr")
                    nc.scalar.activation(lnr[:], pn[:], AF.Ln,
                                         bias=eps_l2[0:1, :])
                    if ti == 0:
                        rqr = rowp.tile([1, 512], F16, name="rqr")
                        with nc.allow_low_precision(reason="1/|q| ~ 0.1"):
                            nc.scalar.activation(rqr[:], lnr[:], AF.Exp,
                                                 bias=0.0, scale=-0.5)
                        nc.gpsimd.partition_broadcast(
                            rqb[:, h * 512:(h + 1) * 512], rqr[:])
                    else:
                        for c in range(4):
                            nc.tensor.matmul(
                                lncol[:, 4 * h + c:4 * h + c + 1],
                                lhsT=lnr[0:1, c * P:(c + 1) * P],
                                rhs=onef[:], start=True, stop=True)
            nc.scalar.activation(rkcol[:, j, :], lncol[:, 0:8], AF.Exp,
                                 bias=ln8_c[:], scale=-0.5)
            nc.vector.tensor_tensor(qT[0:D, js], qT[0:D, js],
                                    rqb[0:D, 0:512], OP.mult)
            nc.vector.tensor_tensor(qT[D:P, js], qT[D:P, js],
                                    rqb[D:P, 512:1024], OP.mult)
            # -- V projection for this j's row blocks --
            for nb in range(4 * j, 4 * j + 4):
                ns = slice(nb * P, (nb + 1) * P)
                pv = psQ.tile([P, 512], F32, name="pq")
                for kt in range(KT):
                    nc.tensor.matmul(pv[:, 0:P], lhsT=xTs[kt][:, ns],
                                     rhs=wq_sb[:, kt, 256:384],
                                     start=(kt == 0), stop=False)
                nc.tensor.matmul(pv[:, 0:P], lhsT=xaug[:, ns],
                                 rhs=waug_sb[:, 256:384],
                                 start=False, stop=True)
                with nc.allow_low_precision(reason="v fp16"):
                    nc.vector.tensor_scalar(
                        out=V[:, nb, 0:D], in0=pv[:, 0:D],
                        scalar1=rsig_col[:, nb:nb + 1], scalar2=None,
                        op0=OP.mult)
                    nc.scalar.activation(V[:, nb, D + 1:2 * D + 1],
                                         pv[:, D:2 * D], AF.Copy, bias=0.0,
                                         scale=rsig_col[:, nb:nb + 1])

        # ---- Phase C: attention + out-projection per megablock ----
        for j in range(QM):
            js = slice(j * 512, (j + 1) * 512)
            KB = 4 * j + 4
            ocat = ocp.tile([P, 512], F16, name="ocat")
            for h in range(2):
                hs = slice(h * D, (h + 1) * D)
                po = psO.tile([D + 1, 512], F32, name="po")
                for kb in range(KB):
                    ps1 = psA.tile([P, 512], F32, name="ps1")
                    nc.tensor.matmul(ps1[:],
                                     lhsT=kT[hs, kb * P:(kb + 1) * P],
                                     rhs=qT[hs, js], start=True, stop=True)
                    E1 = epool.tile([P, 512], F16, name="E1")
                    # exp(8 * qhat.khat): 8/|k| rides the per-partition
                    # scale operand, so kT is consumed unnormalized.
                    kscale = rkcol[:, kb // 4, 4 * h + kb % 4:
                                   4 * h + kb % 4 + 1]
                    q0 = (kb - 4 * j) * P
                    if q0 < 0:
                        nc.scalar.activation(E1[:], ps1[:], AF.Exp,
                                             bias=zero_c[:], scale=kscale)
                    else:
                        nc.scalar.activation(E1[:, q0:512], ps1[:, q0:512],
                                             AF.Exp, bias=zero_c[:],
                                             scale=kscale)
                        nc.gpsimd.affine_select(
                            out=E1[:, 0:q0 + P], in_=E1[:, 0:q0 + P],
                            compare_op=OP.is_ge, fill=0.0, base=-q0,
                            pattern=[[1, q0 + P]], channel_multiplier=-1)
                    nc.tensor.matmul(
                        po[:],
                        lhsT=V[:, kb, h * (D + 1):(h + 1) * (D + 1)],
                        rhs=E1[:], start=(kb == 0), stop=(kb == KB - 1))
                # denominator: recip of PV row 64, DRAM round trip to a
                # [64,512] broadcast, folded into the PSUM->SBUF copy.
                rl = rlp.tile([1, 512], F32, name="rl")
                nc.vector.reciprocal(rl[0:1, :], po[D:D + 1, :])
                rlb = rlp.tile([D, 512], F32, name="rlb")
                nc.gpsimd.partition_broadcast(rlb[:], rl[:])
                with nc.allow_low_precision(reason="o/L fp16"):
                    nc.vector.tensor_tensor(ocat[h * D:(h + 1) * D, :],
                                            po[0:D, :], rlb[0:D, :],
                                            OP.mult)
            # -- out-projection for this megablock --
            for qb in range(4):
                row = (j * 4 + qb) * P
                yt = outp.tile([P, DIM], F16, name="yt")
                for half in range(2):
                    ds = slice(half * 512, (half + 1) * 512)
                    py = psY.tile([P, 512], F32, name="py")
                    nc.tensor.matmul(py[:],
                                     lhsT=ocat[:, qb * P:(qb + 1) * P],
                                     rhs=wout_sb[:, ds],
                                     start=True, stop=True)
                    with nc.allow_low_precision(reason="y fp16 partial"):
                        nc.vector.tensor_copy(yt[:, ds], py[:])
                nc.sync.dma_start(y_d[row:row + P, :], yt[:])

    nc.compile()
    return nc


def make_in_maps(x, ln_w, ln_b, w_qkv, w_out):
    x = np.asarray(x, np.float32)
    ln_w = np.asarray(ln_w, np.float32)
    ln_b = np.asarray(ln_b, np.float32)
    w_qkv = np.asarray(w_qkv, np.float32)
    w_out = np.asarray(w_out, np.float32)
    in_maps = []
    for core in range(N_CORES):
        b, h0 = core // 4, (core % 4) * 2
        cs = [slice(base + h0 * D, base + (h0 + 2) * D)
              for base in (0, 512, 1024)]
        w_parts = [w_qkv[:, c] * ln_w[:, None] for c in cs]
        wcat32 = np.concatenate(w_parts, axis=1)
        wcat = wcat32.astype(np.float16)
        wcat = np.ascontiguousarray(
            wcat.reshape(KT, P, 384).transpose(1, 0, 2))
        colsum = wcat32.sum(axis=0)
        biases = np.concatenate([ln_b @ w_qkv[:, c] for c in cs])
        waug = np.stack([colsum, biases]).astype(np.float16)
        in_maps.append({
            "xT": np.ascontiguousarray(x[b].T.astype(np.float16)),
            "wqkv": wcat,
            "waug": np.ascontiguousarray(waug),
            "wout": np.ascontiguousarray(
                w_out[h0 * D:(h0 + 2) * D]).astype(np.float16),
        })
    return in_maps


def kernel(x, ln_w, ln_b, w_qkv, w_out):
    if "nc" not in _CACHE:
        _CACHE["nc"] = build_nc()
    nc = _CACHE["nc"]
    in_maps = make_in_maps(x, ln_w, ln_b, w_qkv, w_out)
    res = bass_utils.run_bass_kernel_spmd(nc, in_maps,
                                          core_ids=list(range(N_CORES)))
    y = np.zeros((B, N, DIM), np.float32)
    for core in range(N_CORES):
        y[core // 4] += res.results[core]["y"].astype(np.float32)
    return y
